# revision 1
# baseline (speedup 1.0000x reference)
"""Trainium2 Bass kernel for nn_Decoder (GRU decoder, B=64, T_FC=48, C=4096, HID=64).

Strategy
--------
Data-parallel over batch: 8 cores x 8 batch rows -> 32768 independent GRU
"columns" per core (batch*city on the free dim, features on partitions).

Host-side algebra folds fc_in and the autoregressive x_prev feedback into the
gate weights:
    G  = W_ih @ W_in                      [192, 4]
    gates_t = (W_hh + G[:,0:1] @ W_out) @ h_t + G[:,1:4] @ xt_t + const   (t>=1)
(with i_n / h_n kept separate for the r * h_n product).

Layout per 512-column chunk: hidden state lives in one [128, CHUNK] tile
(rows 0:64 = even-step h, 64:128 = odd-step h), which makes the output
projection a K=128 matmul covering two steps at once.  Biases enter through
activation bias vectors and a fused scalar_tensor_tensor.
"""

import os

import numpy as np

import concourse.bass as bass
import concourse.mybir as mybir
import concourse.tile as tile
from concourse import bacc
from concourse.bass_utils import run_bass_kernel_spmd

F32 = mybir.dt.float32
BF16 = mybir.dt.bfloat16
AF = mybir.ActivationFunctionType
ALU = mybir.AluOpType

B, T_HIST, T_FC, C, F_IN, HID = 64, 24, 48, 4096, 8, 64
N_CORES = 8
B_LOC = B // N_CORES
NCOLS = B_LOC * C  # 32768 columns per core
CHUNK = 512

_BUILT = {}
LAST_RESULTS = None  # BassKernelResults of the most recent run (for test.py)

W_SHAPES = {
    # h-side weights duplicated across both partition halves so odd steps
    # (h at rows 64:128) can use a matching lhsT base partition.
    "LRZ_H0": [128, 128], "LRZ_H1": [128, 128],
    "LN_H0": [128, 128], "LN_H1": [128, 128],
    "LRZ_X0": [4, 128], "LN_X0": [4, 128],
    "LRZ_X1": [3, 128], "LN_X1": [3, 128],
    "WOUT2": [128, 2],
    "BRZ0": [128, 1], "BRZ1": [128, 1],
    # per-partition bias vectors duplicated across both halves so either
    # parity's partition base reads the same values
    "BN0": [128, 1], "BN1": [128, 1],
    "BHHN": [128, 1], "BOUT2": [2, 1],
}


def _build(ncols, t_fc):
    key = (ncols, t_fc)
    if key in _BUILT:
        return _BUILT[key]

    nc = bacc.Bacc("TRN2", target_bir_lowering=False, debug=False,
                   num_devices=N_CORES)

    # XTD[k, t, col]: k<3 = decoder exogenous features for step t;
    # k=3 = xn at t=0 (zeros elsewhere).
    d_xtd = nc.dram_tensor("XTD", [4, t_fc, ncols], BF16,
                           kind="ExternalInput").ap()
    d_ht = nc.dram_tensor("HT", [HID, ncols], BF16, kind="ExternalInput").ap()
    d_w = {name: nc.dram_tensor(name, shape,
                                F32 if name.startswith("B") else BF16,
                                kind="ExternalInput").ap()
           for name, shape in W_SHAPES.items()}
    d_out = nc.dram_tensor("OUT", [t_fc, ncols], F32, kind="ExternalOutput").ap()

    nchunks = ncols // CHUNK

    with tile.TileContext(nc) as tc:
        with (
            tc.tile_pool(name="wpool", bufs=1) as wpool,
            tc.tile_pool(name="xpool", bufs=1) as xpool,
            tc.tile_pool(name="hpool", bufs=2) as hpool,
            tc.tile_pool(name="tpool", bufs=8) as tpool,
            tc.tile_pool(name="pspool", bufs=1, space="PSUM") as pspool,
        ):
            w = {}
            for name, ap in d_w.items():
                wt = wpool.tile(list(ap.shape), ap.dtype, name=f"w_{name}")
                nc.gpsimd.dma_start(wt[:], ap[:])
                w[name] = wt

            IL = 8       # chunks processed in lockstep
            XB = 4       # xt steps per DMA block
            for g in range(0, nchunks, IL):
                group = list(range(g, min(g + IL, nchunks)))
                st = {}
                for ci in group:
                    cs = slice(ci * CHUNK, (ci + 1) * CHUNK)
                    xt0 = xpool.tile([4, CHUNK], BF16, tag="xt0",
                                     bufs=IL + 2)
                    nc.gpsimd.dma_start(xt0[:], d_xtd[:, 0, cs])
                    hpair = hpool.tile([128, CHUNK], BF16, tag="hpair",
                                       bufs=IL + 2)
                    nc.gpsimd.dma_start(hpair[0:HID, :], d_ht[:, cs])
                    st[ci] = {"cs": cs, "xt0": xt0, "hpair": hpair,
                              "xtb": None}

                for t in range(t_fc):
                  for ci in group:
                    cs = st[ci]["cs"]
                    hpair = st[ci]["hpair"]
                    if t % XB == 0 and t + 1 < t_fc:
                        # exogenous features for steps t..t+XB-1
                        xtb = xpool.tile([3, XB, CHUNK], BF16, tag="xtb",
                                         bufs=3 * IL)
                        nc.gpsimd.dma_start(
                            xtb[:], d_xtd[0:3, t:t + XB, cs])
                        st[ci]["xtb"] = xtb
                    rb = (t % 2) * HID          # row base of h^(t)
                    wb = HID - rb               # row base of h^(t+1)
                    cur = hpair[rb:rb + HID, :]
                    if t == 0:
                        lrz_h, ln_h = w["LRZ_H0"], w["LN_H0"]
                        lrz_x, ln_x = w["LRZ_X0"], w["LN_X0"]
                        brz, bn = w["BRZ0"], w["BN0"]
                        xt_rhs = st[ci]["xt0"][0:4, :]
                    else:
                        lrz_h, ln_h = w["LRZ_H1"], w["LN_H1"]
                        lrz_x, ln_x = w["LRZ_X1"], w["LN_X1"]
                        brz, bn = w["BRZ1"], w["BN1"]
                        xt_rhs = st[ci]["xtb"][0:3, t % XB, :]

                    rzp = pspool.tile([128, CHUNK], F32, tag="rz", bufs=3)
                    npp = pspool.tile([128, CHUNK], F32, tag="n", bufs=3)
                    lrz_hs = lrz_h[rb:rb + HID, :]
                    ln_hs = ln_h[rb:rb + HID, :]
                    nc.tensor.matmul(rzp[:], lrz_hs, cur, start=True,
                                     stop=False)
                    nc.tensor.matmul(rzp[:], lrz_x[:], xt_rhs, start=False,
                                     stop=True)
                    nc.tensor.matmul(npp[:], ln_hs, cur, start=True,
                                     stop=False)
                    nc.tensor.matmul(npp[:], ln_x[:], xt_rhs, start=False,
                                     stop=True)

                    # [z; r] = sigmoid(rz psum + bias)  (z rows 0:64)
                    rzs = tpool.tile([128, CHUNK], BF16, tag="rzs")
                    nc.scalar.activation(rzs[:], rzp[:], AF.Sigmoid,
                                         bias=brz[:])
                    # Evacuate [i_n; h_n] psum in one ACT op; bias vector
                    # adds b_hh_n to the h_n half only.
                    nsb = tpool.tile([128, CHUNK], BF16, tag="nsb")
                    nc.scalar.activation(nsb[:], npp[:], AF.Identity,
                                         bias=w["BHHN"][:])
                    sl = slice(rb, rb + HID)
                    # r * (h_n + b_hh_n)   (both operands at base 64)
                    rhn = tpool.tile([128, CHUNK], BF16, tag="rhn")
                    nc.vector.tensor_tensor(rhn[0:HID, :], nsb[HID:128, :],
                                            rzs[HID:128, :], op=ALU.mult)
                    # i_n + r*h_n          (both at base 0)
                    npre = tpool.tile([128, CHUNK], BF16, tag="npre")
                    nc.vector.tensor_tensor(npre[0:HID, :], rhn[0:HID, :],
                                            nsb[0:HID, :], op=ALU.add)
                    nt = tpool.tile([128, CHUNK], BF16, tag="nt")
                    nc.scalar.activation(nt[sl, :], npre[0:HID, :], AF.Tanh,
                                         bias=bn[0:HID, :])
                    # h' = n + z*(h - n)
                    hm = tpool.tile([128, CHUNK], BF16, tag="hm")
                    nc.vector.tensor_tensor(hm[0:HID, :], cur, nt[sl, :],
                                            op=ALU.subtract)
                    zt = tpool.tile([128, CHUNK], BF16, tag="zt")
                    nc.vector.tensor_tensor(zt[sl, :], rzs[0:HID, :],
                                            hm[0:HID, :], op=ALU.mult)
                    nc.vector.tensor_tensor(hpair[wb:wb + HID, :], nt[sl, :],
                                            zt[sl, :], op=ALU.add)

                    if t % 2 == 1:
                        # [pred_{t-1}; pred_t] = WOUT2.T @ [h^(t+1); h^(t)]
                        pp = pspool.tile([2, CHUNK], F32, tag="pred", bufs=2)
                        nc.tensor.matmul(pp[:], w["WOUT2"][:], hpair[:],
                                         start=True, stop=True)
                        pst = tpool.tile([2, CHUNK], F32, tag="pst")
                        nc.scalar.add(pst[:], pp[:], w["BOUT2"][:])
                        nc.gpsimd.dma_start(d_out[t - 1:t + 1, cs], pst[:])

    nc.compile()
    _BUILT[key] = nc
    return nc


def _prep_weights(W_in, b_in, W_ih, W_hh, b_ih, b_hh, W_out, b_out):
    f8 = np.float64
    G = W_ih.astype(f8) @ W_in.astype(f8)              # [192, 4]
    c = W_ih.astype(f8) @ b_in.astype(f8) + b_ih       # [192]
    wo = W_out.astype(f8)[0]                           # [64]
    bo = float(b_out[0])
    A1 = W_hh.astype(f8) + np.outer(G[:, 0], wo)       # [192, 64]
    d0 = c + b_hh                                      # [192]
    d1 = d0 + G[:, 0] * bo

    def dup(m):  # duplicate across both partition halves
        return np.concatenate([m, m], axis=0)

    def rzswap(m):  # [*,128] gate cols: [r;z] -> [z;r]
        return np.concatenate([m[:, HID:128], m[:, 0:HID]], axis=1)

    w = {}
    w["LRZ_H1"] = dup(rzswap(A1[:128].T))
    w["LRZ_H0"] = dup(rzswap(W_hh[:128].astype(f8).T))

    ln_h1 = np.zeros((HID, 128), f8)
    ln_h1[:, 0:HID] = np.outer(wo, G[128:, 0])         # i_n feedback
    ln_h1[:, HID:128] = W_hh[128:].astype(f8).T        # h_n
    w["LN_H1"] = dup(ln_h1)
    ln_h0 = np.zeros((HID, 128), f8)
    ln_h0[:, HID:128] = W_hh[128:].astype(f8).T
    w["LN_H0"] = dup(ln_h0)

    w["LRZ_X1"] = rzswap(G[:128, 1:4].T)
    ln_x1 = np.zeros((3, 128), f8)
    ln_x1[:, 0:HID] = G[128:, 1:4].T
    w["LN_X1"] = ln_x1

    # step-0 rhs row order is [xt1, xt2, xt3, xn]
    perm = [1, 2, 3, 0]
    w["LRZ_X0"] = rzswap(G[:128, perm].T)
    ln_x0 = np.zeros((4, 128), f8)
    ln_x0[:, 0:HID] = G[128:, perm].T
    w["LN_X0"] = ln_x0

    wout2 = np.zeros((128, 2), f8)
    wout2[HID:128, 0] = wo                  # pred_{t-1} from odd rows h^(t)
    wout2[0:HID, 1] = wo                    # pred_t from even rows h^(t+1)
    w["WOUT2"] = wout2

    def rzswap_v(v):
        return np.concatenate([v[HID:128], v[0:HID]], axis=0)

    w["BRZ0"] = rzswap_v(d0[:128])[:, None]
    w["BRZ1"] = rzswap_v(d1[:128])[:, None]
    w["BN0"] = dup(c[128:, None])
    w["BN1"] = dup((c[128:] + G[128:, 0] * bo)[:, None])
    bhhn = np.zeros((128, 1), f8)
    bhhn[HID:128, 0] = b_hh[128:]
    w["BHHN"] = bhhn
    w["BOUT2"] = np.full((2, 1), bo, f8)

    import ml_dtypes
    return {k: np.ascontiguousarray(
        v.astype(np.float32 if k.startswith("B") else ml_dtypes.bfloat16))
        for k, v in w.items()}


def kernel(X, H, xn, W_in, b_in, W_ih, W_hh, b_ih, b_hh, W_out, b_out):
    global LAST_RESULTS
    X = np.asarray(X, np.float32)
    H = np.asarray(H, np.float32)
    xn = np.asarray(xn, np.float32)
    wmap = _prep_weights(np.asarray(W_in), np.asarray(b_in), np.asarray(W_ih),
                         np.asarray(W_hh), np.asarray(b_ih), np.asarray(b_hh),
                         np.asarray(W_out), np.asarray(b_out))

    Xs = X[:, T_HIST:T_HIST + T_FC, :, F_IN - 3:F_IN]  # [B, 48, C, 3]

    in_maps = []
    for ci in range(N_CORES):
        bs = slice(ci * B_LOC, (ci + 1) * B_LOC)
        Xc = Xs[bs]                                     # [8, 48, C, 3]
        import ml_dtypes
        XTD = np.zeros((4, T_FC, NCOLS), ml_dtypes.bfloat16)
        XTD[0:3] = np.transpose(Xc, (3, 1, 0, 2)).reshape(3, T_FC, NCOLS)
        XTD[3, 0] = xn[bs, :, 0].reshape(NCOLS)
        HT = np.ascontiguousarray(
            H[bs].transpose(2, 0, 1).reshape(HID, NCOLS).astype(ml_dtypes.bfloat16))
        m = {"XTD": XTD, "HT": HT}
        m.update(wmap)
        in_maps.append(m)

    nc = _build(NCOLS, T_FC)

    trace = os.environ.get("BASS_KERNEL_TRACE") == "1"
    if trace:
        _register_ntff_hook()
    res = run_bass_kernel_spmd(nc, in_maps, list(range(N_CORES)), trace=trace)
    LAST_RESULTS = res

    out = np.empty((B, T_FC, C, 1), np.float32)
    for ci in range(N_CORES):
        o = res.results[ci]["OUT"].reshape(T_FC, B_LOC, C)
        out[ci * B_LOC:(ci + 1) * B_LOC] = o.transpose(1, 0, 2)[..., None]
    return out


def _register_ntff_hook():
    """The agent image's antenv lacks axon_hooks; provide it so trace=True
    can capture NTFF profiles through libaxon_pjrt."""
    import sys
    import types
    if "antenv.axon_hooks" in sys.modules:
        return
    mod = types.ModuleType("antenv.axon_hooks")
    state = {"hook": None}
    mod.set_axon_ntff_profile_hook = lambda h: state.update(hook=h)
    mod.get_axon_ntff_profile_hook = lambda: state["hook"]
    sys.modules["antenv.axon_hooks"] = mod
    try:
        import antenv
        antenv.axon_hooks = mod
    except ImportError:
        pass
    try:
        from trn_agent_boot.trn_boot import _ntff_profile_via_ctypes
        hook = _ntff_profile_via_ctypes("/opt/axon/libaxon_pjrt.so")
        if hook is not None:
            mod.set_axon_ntff_profile_hook(hook)
    except Exception as e:  # pragma: no cover
        print(f"NTFF hook registration failed: {e}")
    # No artifact bucket in this sandbox; keep profiles local.
    import concourse.bass_utils as bu
    bu.upload_artifacts = lambda tmpdir: f"file://{tmpdir}"



# revision 2
# speedup vs baseline: 1.9810x; 1.9810x over previous
"""Trainium2 Bass kernel for nn_Decoder (GRU decoder, B=64, T_FC=48, C=4096, HID=64).

Strategy (v2)
-------------
Data-parallel over batch: 8 cores x 8 batch rows -> 32768 independent GRU
"columns" per core.  Columns are processed in GROUPS of 1024: chunk A
(cols 0:512) occupies partitions 0:64, chunk B (cols 512:1024) partitions
64:128, so every DVE/ACT op runs with all 128 partitions busy.

Host algebra folds fc_in and the autoregressive x_prev feedback into the
gate weights (t>=1):
    G = W_ih @ W_in                       [192, 4]
    pre_g   = (W_hh_g + G_g0 wo^T) h + G_g,1:4 xt + bias_g     (g in r,z)
    i_n     = (G_n0 wo^T) h + G_n,1:4 xt + bias_n
    h_n     = W_hh_n h            (+ b_hh_n via scalar_tensor_tensor)
    n = tanh(i_n + r*h_n);  h' = n + z*(h - n);  pred = wo @ h' (+ b_out host)

Per group-step: 9 back-to-back N=512 bf16 matmuls (block-diagonal K=128
h-side, K=9 x-side incl. bias/aux rows, identity-matmul accumulating
r*h_n into the i_n psum, sparse pred matmul parking 8 steps of preds in
one psum bank), 1 sigmoid [128,1024], 1 tanh [128,512], and ~4.5 DVE ops
(update ops run per double-group at [128,1024]).  Dense MM stream keeps
the PE at full clock (216ns/MM vs 604ns in the dependency-choked v1).
"""

import os

import numpy as np

import concourse.bass as bass
import concourse.mybir as mybir
import concourse.tile as tile
from concourse import bacc
from concourse.bass_utils import run_bass_kernel_spmd

F32 = mybir.dt.float32
BF16 = mybir.dt.bfloat16
AF = mybir.ActivationFunctionType
ALU = mybir.AluOpType

B, T_HIST, T_FC, C, F_IN, HID = 64, 24, 48, 4096, 8, 64
N_CORES = 8
B_LOC = B // N_CORES
NCOLS = B_LOC * C          # 32768 columns per core
NG = 32                    # groups of 1024 columns
ND = 16                    # double-groups
W_GROUPS = 8               # groups per window (4 double-groups)
XB = 4                     # xt steps per DMA block

_BUILT = {}
LAST_RESULTS = None  # BassKernelResults of the most recent run (for test.py)

W_SHAPES = {
    "TRH0": [128, 128], "TRH1": [128, 128],
    "TZH0": [128, 128], "TZH1": [128, 128],
    "TIH1": [128, 128], "THH": [128, 128], "ID128": [128, 128],
    "XR0": [9, 128], "XR1": [9, 128],
    "XZ0": [9, 128], "XZ1": [9, 128],
    "XI0": [9, 128], "XI1": [9, 128],
    "PW": [128, 32 * 128],
    "BHHN": [128, 1],
}


def _build():
    key = "v2"
    if key in _BUILT:
        return _BUILT[key]

    nc = bacc.Bacc("TRN2", target_bir_lowering=False, debug=False,
                   num_devices=N_CORES)

    d_xtd = nc.dram_tensor("XTD", [ND, 9, T_FC, 1024], BF16,
                           kind="ExternalInput").ap()
    d_ht = nc.dram_tensor("HT", [ND, 128, 1024], BF16,
                          kind="ExternalInput").ap()
    d_w = {name: nc.dram_tensor(name, shape,
                                F32 if name == "BHHN" else BF16,
                                kind="ExternalInput").ap()
           for name, shape in W_SHAPES.items()}
    # preds: [window, bank, 128, 6*512]; row = 32*a + 2*j + chunk
    d_out = nc.dram_tensor("OUT", [NG // W_GROUPS, 2, 128, 3072], BF16,
                           kind="ExternalOutput").ap()

    with tile.TileContext(nc) as tc:
        with (
            tc.tile_pool(name="wpool", bufs=1) as wpool,
            tc.tile_pool(name="xpool", bufs=1) as xpool,
            tc.tile_pool(name="hpool", bufs=1) as hpool,
            tc.tile_pool(name="spool", bufs=1) as spool,
            tc.tile_pool(name="pspool", bufs=1, space="PSUM") as pspool,
        ):
            w = {}
            for name, ap in d_w.items():
                wt = wpool.tile(list(ap.shape), ap.dtype, name=f"w_{name}")
                nc.gpsimd.dma_start(wt[:], ap[:])
                w[name] = wt

            def PWk(a, j):
                k = a * 8 + j
                return w["PW"][:, k * 128:(k + 1) * 128]

            for win in range(NG // W_GROUPS):
                Hd = {}
                Sd = {}
                NTd = {}
                xtb = {}
                for d in range(4):
                    dbl = win * 4 + d
                    ht = hpool.tile([128, 2, 512], BF16, tag=f"H{d}",
                                    bufs=2, name="ht")
                    nc.gpsimd.dma_start(ht[:], d_ht[dbl])
                    Hd[d] = ht
                psb = {}
                for bank in range(2):
                    psb[bank] = spool.tile([128, 3072], BF16,
                                           tag=f"psb{bank}", bufs=2,
                                           name="psb")
                pr = {}
                for t in range(T_FC):
                    if t % 8 == 0:
                        pr[0] = pspool.tile([128, 512], F32, tag="pra",
                                            bufs=1, name="pra")
                        pr[1] = pspool.tile([128, 512], F32, tag="prb",
                                            bufs=1, name="prb")
                    for gi in range(W_GROUPS):
                        d, j2 = gi // 2, gi % 2
                        dbl = win * 4 + d
                        if t % XB == 0 and j2 == 0:
                            xt_ = xpool.tile([9, XB, 1024], BF16,
                                             tag=f"xt{d}", bufs=2, name="xt_")
                            nc.gpsimd.dma_start(
                                xt_[:], d_xtd[dbl, :, t:t + XB, :])
                            xtb[d] = xt_
                        xts = xtb[d][:, t % XB, j2 * 512:(j2 + 1) * 512]
                        hs = Hd[d][:, j2, :]

                        g = pspool.tile([128, 1024], F32, tag="gates",
                                        bufs=3, name="g")
                        trh = w["TRH1"] if t else w["TRH0"]
                        tzh = w["TZH1"] if t else w["TZH0"]
                        xr = w["XR1"] if t else w["XR0"]
                        xz = w["XZ1"] if t else w["XZ0"]
                        xi = w["XI1"] if t else w["XI0"]
                        nc.tensor.matmul(g[:, 0:512], trh[:], hs,
                                         start=True, stop=False)
                        nc.tensor.matmul(g[:, 0:512], xr[:], xts,
                                         start=False, stop=True)
                        nc.tensor.matmul(g[:, 512:1024], tzh[:], hs,
                                         start=True, stop=False)
                        nc.tensor.matmul(g[:, 512:1024], xz[:], xts,
                                         start=False, stop=True)

                        if j2 == 0:
                            Sd[d] = spool.tile([128, 2, 1024], BF16,
                                               tag=f"S{d}", bufs=2, name="S")
                            NTd[d] = spool.tile([128, 2, 512], BF16,
                                                tag=f"NT{d}", bufs=2,
                                                name="NT")
                        nc.scalar.activation(Sd[d][:, j2, :], g[:],
                                             AF.Sigmoid)

                        # phase 2: reuse gate banks for [i_n | h_n]
                        if t:
                            nc.tensor.matmul(g[:, 0:512], w["TIH1"][:], hs,
                                             start=True, stop=False)
                            nc.tensor.matmul(g[:, 0:512], xi[:], xts,
                                             start=False, stop=False)
                        else:
                            nc.tensor.matmul(g[:, 0:512], xi[:], xts,
                                             start=True, stop=False)
                        nc.tensor.matmul(g[:, 512:1024], w["THH"][:], hs,
                                         start=True, stop=True)

                        rhn = spool.tile([128, 512], BF16, tag="rhn",
                                         bufs=3, name="rhn")
                        nc.vector.scalar_tensor_tensor(
                            rhn[:], g[:, 512:1024], w["BHHN"][:],
                            Sd[d][:, j2, 0:512], op0=ALU.add, op1=ALU.mult)
                        nc.tensor.matmul(g[:, 0:512], w["ID128"][:], rhn[:],
                                         start=False, stop=True)
                        nc.scalar.activation(NTd[d][:, j2, :], g[:, 0:512],
                                             AF.Tanh)

                        if j2 == 1:
                            hm = spool.tile([128, 2, 512], BF16,
                                            tag=f"HM{d}", bufs=2, name="hm")
                            nc.vector.tensor_tensor(hm[:], Hd[d][:],
                                                    NTd[d][:],
                                                    op=ALU.subtract)
                            zt = spool.tile([128, 2, 512], BF16,
                                            tag=f"ZT{d}", bufs=2, name="zt")
                            nc.vector.tensor_tensor(zt[:],
                                                    Sd[d][:, :, 512:1024],
                                                    hm[:], op=ALU.mult)
                            nc.vector.tensor_tensor(Hd[d][:], NTd[d][:],
                                                    zt[:], op=ALU.add)
                            # preds of h^{t+1} for both groups of the pair
                            for jj in (0, 1):
                                gidx = 2 * d + jj
                                a, bank = gidx % 4, gidx // 4
                                nc.tensor.matmul(
                                    pr[bank][:], PWk(a, t % 8),
                                    Hd[d][:, jj, :],
                                    start=(t % 8 == 0 and a == 0),
                                    stop=(t % 8 == 7 and a == 3),
                                    skip_group_check=True)
                    if t % 8 == 7:
                        blk = t // 8
                        for bank in range(2):
                            nc.vector.tensor_copy(
                                psb[bank][:, blk * 512:(blk + 1) * 512],
                                pr[bank][:])
                for bank in range(2):
                    nc.gpsimd.dma_start(d_out[win, bank], psb[bank][:])

    nc.compile()
    _BUILT[key] = nc
    return nc


def _prep_weights(W_in, b_in, W_ih, W_hh, b_ih, b_hh, W_out, b_out):
    f8 = np.float64
    G = W_ih.astype(f8) @ W_in.astype(f8)              # [192, 4]
    c = W_ih.astype(f8) @ b_in.astype(f8) + b_ih       # [192]
    wo = W_out.astype(f8)[0]                           # [64]
    bo = float(b_out[0])
    Wr, Wz, Wn = W_hh[0:64].astype(f8), W_hh[64:128].astype(f8), \
        W_hh[128:192].astype(f8)
    Gr, Gz, Gn = G[0:64], G[64:128], G[128:192]
    cr, cz, cn = c[0:64], c[64:128], c[128:192]
    bhr, bhz, bhn = b_hh[0:64].astype(f8), b_hh[64:128].astype(f8), \
        b_hh[128:192].astype(f8)

    def blockdiag(m):  # [64,64] effective weight -> [128,128] lhsT
        out = np.zeros((128, 128), f8)
        out[0:64, 0:64] = m.T
        out[64:128, 64:128] = m.T
        return out

    def xlhs(Gg, bias):  # [9, 128] x-side lhsT
        out = np.zeros((9, 128), f8)
        out[0:3, 0:64] = Gg[:, 1:4].T
        out[3:6, 64:128] = Gg[:, 1:4].T
        out[6, 0:64] = bias
        out[6, 64:128] = bias
        out[7, 0:64] = Gg[:, 0]
        out[8, 64:128] = Gg[:, 0]
        return out

    w = {}
    w["TRH0"] = blockdiag(Wr)
    w["TRH1"] = blockdiag(Wr + np.outer(Gr[:, 0], wo))
    w["TZH0"] = blockdiag(Wz)
    w["TZH1"] = blockdiag(Wz + np.outer(Gz[:, 0], wo))
    w["TIH1"] = blockdiag(np.outer(Gn[:, 0], wo))
    w["THH"] = blockdiag(Wn)
    w["ID128"] = np.eye(128, dtype=f8)
    w["XR0"] = xlhs(Gr, cr + bhr)
    w["XR1"] = xlhs(Gr, cr + bhr + Gr[:, 0] * bo)
    w["XZ0"] = xlhs(Gz, cz + bhz)
    w["XZ1"] = xlhs(Gz, cz + bhz + Gz[:, 0] * bo)
    w["XI0"] = xlhs(Gn, cn)
    w["XI1"] = xlhs(Gn, cn + Gn[:, 0] * bo)
    pw = np.zeros((128, 32 * 128), f8)
    for a in range(4):
        for j in range(8):
            k = a * 8 + j
            col = 32 * a + 2 * j
            pw[0:64, k * 128 + col] = wo
            pw[64:128, k * 128 + col + 1] = wo
    w["PW"] = pw
    w["BHHN"] = np.concatenate([bhn, bhn])[:, None]

    import ml_dtypes
    return {k: np.ascontiguousarray(
        v.astype(np.float32 if k == "BHHN" else ml_dtypes.bfloat16))
        for k, v in w.items()}


def kernel(X, H, xn, W_in, b_in, W_ih, W_hh, b_ih, b_hh, W_out, b_out):
    global LAST_RESULTS
    import ml_dtypes
    X = np.asarray(X, np.float32)
    H = np.asarray(H, np.float32)
    xn = np.asarray(xn, np.float32)
    bo = float(np.asarray(b_out)[0])
    wmap = _prep_weights(np.asarray(W_in), np.asarray(b_in), np.asarray(W_ih),
                         np.asarray(W_hh), np.asarray(b_ih), np.asarray(b_hh),
                         np.asarray(W_out), np.asarray(b_out))

    Xs = X[:, T_HIST:T_HIST + T_FC, :, F_IN - 3:F_IN]  # [B, 48, C, 3]

    in_maps = []
    for ci in range(N_CORES):
        bs = slice(ci * B_LOC, (ci + 1) * B_LOC)
        # [t, f, col], col = b*C + c
        r3 = np.transpose(Xs[bs], (1, 3, 0, 2)).reshape(T_FC, 3, NCOLS)
        r6 = r3.reshape(T_FC, 3, ND, 2, 2, 512)  # [t,f,dbl,odd,ab,col]
        XTD = np.zeros((ND, 9, T_FC, 1024), ml_dtypes.bfloat16)
        XTD[:, 0:3] = r6[:, :, :, :, 0, :].transpose(2, 1, 0, 3, 4) \
            .reshape(ND, 3, T_FC, 1024)
        XTD[:, 3:6] = r6[:, :, :, :, 1, :].transpose(2, 1, 0, 3, 4) \
            .reshape(ND, 3, T_FC, 1024)
        XTD[:, 6] = 1.0
        xn6 = xn[bs, :, 0].reshape(ND, 2, 2, 512)
        XTD[:, 7, 0, :] = xn6[:, :, 0, :].reshape(ND, 1024)
        XTD[:, 8, 0, :] = xn6[:, :, 1, :].reshape(ND, 1024)

        h6 = H[bs].reshape(NCOLS, HID).reshape(ND, 2, 2, 512, HID)
        HT = np.empty((ND, 128, 1024), ml_dtypes.bfloat16)
        HT[:, 0:64] = h6[:, :, 0].transpose(0, 3, 1, 2).reshape(ND, 64, 1024)
        HT[:, 64:128] = h6[:, :, 1].transpose(0, 3, 1, 2) \
            .reshape(ND, 64, 1024)

        m = {"XTD": XTD, "HT": np.ascontiguousarray(HT)}
        m.update(wmap)
        in_maps.append(m)

    nc = _build()

    trace = os.environ.get("BASS_KERNEL_TRACE") == "1"
    if trace:
        _register_ntff_hook()
    res = run_bass_kernel_spmd(nc, in_maps, list(range(N_CORES)), trace=trace)
    LAST_RESULTS = res

    out = np.empty((B, T_FC, C, 1), np.float32)
    for ci in range(N_CORES):
        O = np.asarray(res.results[ci]["OUT"], np.float32)
        # [w, bank, a, q(32), blk, col] -> keep q<16 -> [w,bank,a,j,ch,blk,col]
        O7 = O.reshape(4, 2, 4, 32, 6, 512)[:, :, :, 0:16] \
            .reshape(4, 2, 4, 8, 2, 6, 512)
        # -> [blk, j, w, bank, a, ch, col] = [t, row]
        P = O7.transpose(5, 3, 0, 1, 2, 4, 6).reshape(T_FC, NCOLS) + bo
        out[ci * B_LOC:(ci + 1) * B_LOC] = \
            P.reshape(T_FC, B_LOC, C).transpose(1, 0, 2)[..., None]
    return out


def _register_ntff_hook():
    """The agent image's antenv lacks axon_hooks; provide it so trace=True
    can capture NTFF profiles through libaxon_pjrt."""
    import sys
    import types
    if "antenv.axon_hooks" in sys.modules:
        return
    mod = types.ModuleType("antenv.axon_hooks")
    state = {"hook": None}
    mod.set_axon_ntff_profile_hook = lambda h: state.update(hook=h)
    mod.get_axon_ntff_profile_hook = lambda: state["hook"]
    sys.modules["antenv.axon_hooks"] = mod
    try:
        import antenv
        antenv.axon_hooks = mod
    except ImportError:
        pass
    try:
        from trn_agent_boot.trn_boot import _ntff_profile_via_ctypes
        hook = _ntff_profile_via_ctypes("/opt/axon/libaxon_pjrt.so")
        if hook is not None:
            mod.set_axon_ntff_profile_hook(hook)
    except Exception as e:  # pragma: no cover
        print(f"NTFF hook registration failed: {e}")
    # No artifact bucket in this sandbox; keep profiles local.
    import concourse.bass_utils as bu
    bu.upload_artifacts = lambda tmpdir: f"file://{tmpdir}"


# revision 6
# speedup vs baseline: 2.4117x; 1.2174x over previous
"""Trainium2 Bass kernel for nn_Decoder (GRU decoder, B=64, T_FC=48, C=4096, HID=64).

Strategy (v2)
-------------
Data-parallel over batch: 8 cores x 8 batch rows -> 32768 independent GRU
"columns" per core.  Columns are processed in GROUPS of 1024: chunk A
(cols 0:512) occupies partitions 0:64, chunk B (cols 512:1024) partitions
64:128, so every DVE/ACT op runs with all 128 partitions busy.

Host algebra folds fc_in and the autoregressive x_prev feedback into the
gate weights (t>=1):
    G = W_ih @ W_in                       [192, 4]
    pre_g   = (W_hh_g + G_g0 wo^T) h + G_g,1:4 xt + bias_g     (g in r,z)
    i_n     = (G_n0 wo^T) h + G_n,1:4 xt + bias_n
    h_n     = W_hh_n h            (+ b_hh_n via scalar_tensor_tensor)
    n = tanh(i_n + r*h_n);  h' = n + z*(h - n);  pred = wo @ h' (+ b_out host)

Per group-step: 9 back-to-back N=512 bf16 matmuls (block-diagonal K=128
h-side, K=9 x-side incl. bias/aux rows, identity-matmul accumulating
r*h_n into the i_n psum, sparse pred matmul parking 8 steps of preds in
one psum bank), 1 sigmoid [128,1024], 1 tanh [128,512], and ~4.5 DVE ops
(update ops run per double-group at [128,1024]).  Dense MM stream keeps
the PE at full clock (216ns/MM vs 604ns in the dependency-choked v1).
"""

import os

import numpy as np

import concourse.bass as bass
import concourse.mybir as mybir
import concourse.tile as tile
from concourse import bacc
from concourse.bass_utils import run_bass_kernel_spmd

F32 = mybir.dt.float32
BF16 = mybir.dt.bfloat16
AF = mybir.ActivationFunctionType
ALU = mybir.AluOpType

B, T_HIST, T_FC, C, F_IN, HID = 64, 24, 48, 4096, 8, 64
N_CORES = 8
B_LOC = B // N_CORES
NCOLS = B_LOC * C          # 32768 columns per core
NG = 32                    # groups of 1024 columns
ND = 16                    # double-groups
W_GROUPS = 8               # groups per window (4 double-groups)
XQ = 12                    # xt steps packed per [128,1024] tile

_BUILT = {}
LAST_RESULTS = None  # BassKernelResults of the most recent run (for test.py)

W_SHAPES = {
    "TRH0": [128, 128], "TRH1": [128, 128],
    "TZH0": [128, 128], "TZH1": [128, 128],
    "TIH1": [128, 128], "THH": [128, 128], "ID128": [128, 128],
    "XR0P": [128, 128], "XZ0P": [128, 128], "XI0P": [128, 128],
    "XRP": [128, 12 * 128], "XZP": [128, 12 * 128], "XIP": [128, 12 * 128],
    "PW": [128, 32 * 128],
    "BHHN": [128, 1],
}


def _build():
    key = "v2"
    if key in _BUILT:
        return _BUILT[key]

    nc = bacc.Bacc("TRN2", target_bir_lowering=False, debug=False,
                   num_devices=N_CORES)

    d_xtd = nc.dram_tensor("XTD", [ND, 4, 128, 1024], BF16,
                           kind="ExternalInput").ap()
    d_ht = nc.dram_tensor("HT", [ND, 128, 1024], BF16,
                          kind="ExternalInput").ap()
    d_w = {name: nc.dram_tensor(name, shape,
                                F32 if name == "BHHN" else BF16,
                                kind="ExternalInput").ap()
           for name, shape in W_SHAPES.items()}
    # preds: [window, bank, 128, 6*512]; row = 32*a + 2*j + chunk
    d_out = nc.dram_tensor("OUT", [NG // W_GROUPS, 2, 128, 3072], BF16,
                           kind="ExternalOutput").ap()

    with tile.TileContext(nc) as tc:
        with (
            tc.tile_pool(name="wpool", bufs=1) as wpool,
            tc.tile_pool(name="xpool", bufs=1) as xpool,
            tc.tile_pool(name="hpool", bufs=1) as hpool,
            tc.tile_pool(name="spool", bufs=1) as spool,
            tc.tile_pool(name="pspool", bufs=1, space="PSUM") as pspool,
        ):
            w = {}
            for name, ap in d_w.items():
                wt = wpool.tile(list(ap.shape), ap.dtype, name=f"w_{name}")
                nc.gpsimd.dma_start(wt[:], ap[:])
                w[name] = wt

            def PWk(a, j):
                k = a * 8 + j
                return w["PW"][:, k * 128:(k + 1) * 128]

            def XWq(name, q):
                return w[name][:, q * 128:(q + 1) * 128]

            for win in range(NG // W_GROUPS):
                Hd = {}
                Sd = {}
                NTd = {}
                xtb = {}
                for d in range(4):
                    dbl = win * 4 + d
                    ht = hpool.tile([128, 2, 512], BF16, tag=f"H{d}",
                                    bufs=2, name="ht")
                    nc.gpsimd.dma_start(ht[:], d_ht[dbl])
                    Hd[d] = ht
                psb = {}
                for bank in range(2):
                    psb[bank] = spool.tile([128, 3072], BF16,
                                           tag=f"psb{bank}", bufs=2,
                                           name="psb")
                pr = {}
                for t in range(T_FC):
                    if t % 8 == 0:
                        pr[0] = pspool.tile([128, 512], F32, tag="pra",
                                            bufs=1, name="pra")
                        pr[1] = pspool.tile([128, 512], F32, tag="prb",
                                            bufs=1, name="prb")
                    trh = w["TRH1"] if t else w["TRH0"]
                    tzh = w["TZH1"] if t else w["TZH0"]
                    q = t % XQ
                    xr = w["XR0P"] if t == 0 else XWq("XRP", q)
                    xz = w["XZ0P"] if t == 0 else XWq("XZP", q)
                    xi = w["XI0P"] if t == 0 else XWq("XIP", q)
                    for gi in range(W_GROUPS):
                        d, j2 = gi // 2, gi % 2
                        dbl = win * 4 + d
                        if t % XQ == 0 and j2 == 0:
                            xt_ = xpool.tile([128, 1024], BF16,
                                             tag=f"xt{d}", bufs=2, name="xt_")
                            nc.gpsimd.dma_start(
                                xt_[:], d_xtd[dbl, t // XQ])
                            xtb[d] = xt_
                        xts = xtb[d][:, j2 * 512:(j2 + 1) * 512]
                        hs = Hd[d][:, j2, :]

                        g = pspool.tile([128, 1024], F32, tag="gates",
                                        bufs=3, name="g")
                        nc.tensor.matmul(g[:, 0:512], trh[:], hs,
                                         start=True, stop=False)
                        nc.tensor.matmul(g[:, 0:512], xr[:], xts,
                                         start=False, stop=True)
                        nc.tensor.matmul(g[:, 512:1024], tzh[:], hs,
                                         start=True, stop=False)
                        nc.tensor.matmul(g[:, 512:1024], xz[:], xts,
                                         start=False, stop=True)

                        if j2 == 0:
                            Sd[d] = spool.tile([128, 2, 1024], BF16,
                                               tag=f"S{d}", bufs=2, name="S")
                            NTd[d] = spool.tile([128, 2, 512], BF16,
                                                tag=f"NT{d}", bufs=2,
                                                name="NT")
                        nc.scalar.activation(Sd[d][:, j2, :], g[:],
                                             AF.Sigmoid)

                        # phase 2: reuse gate banks for [i_n | h_n]
                        if t:
                            nc.tensor.matmul(g[:, 0:512], w["TIH1"][:], hs,
                                             start=True, stop=False)
                            nc.tensor.matmul(g[:, 0:512], xi[:], xts,
                                             start=False, stop=False)
                        else:
                            nc.tensor.matmul(g[:, 0:512], xi[:], xts,
                                             start=True, stop=False)
                        nc.tensor.matmul(g[:, 512:1024], w["THH"][:], hs,
                                         start=True, stop=True)

                        rhn = spool.tile([128, 512], BF16, tag="rhn",
                                         bufs=3, name="rhn")
                        nc.vector.scalar_tensor_tensor(
                            rhn[:], g[:, 512:1024], w["BHHN"][:],
                            Sd[d][:, j2, 0:512], op0=ALU.add, op1=ALU.mult)
                        nc.tensor.matmul(g[:, 0:512], w["ID128"][:], rhn[:],
                                         start=False, stop=True)
                        nc.scalar.activation(NTd[d][:, j2, :], g[:, 0:512],
                                             AF.Tanh)

                        if j2 == 1:
                            hm = spool.tile([128, 2, 512], BF16,
                                            tag=f"HM{d}", bufs=2, name="hm")
                            nc.vector.tensor_tensor(hm[:], Hd[d][:],
                                                    NTd[d][:],
                                                    op=ALU.subtract)
                            zt = spool.tile([128, 2, 512], BF16,
                                            tag=f"ZT{d}", bufs=2, name="zt")
                            nc.vector.tensor_tensor(zt[:],
                                                    Sd[d][:, :, 512:1024],
                                                    hm[:], op=ALU.mult)
                            nc.vector.tensor_tensor(Hd[d][:], NTd[d][:],
                                                    zt[:], op=ALU.add)
                            # preds of h^{t+1} for both groups of the pair
                            for jj in (0, 1):
                                gidx = 2 * d + jj
                                a, bank = gidx % 4, gidx // 4
                                nc.tensor.matmul(
                                    pr[bank][:], PWk(a, t % 8),
                                    Hd[d][:, jj, :],
                                    start=(t % 8 == 0 and a == 0),
                                    stop=(t % 8 == 7 and a == 3),
                                    skip_group_check=True)
                    if t % 8 == 7:
                        blk = t // 8
                        for bank in range(2):
                            nc.vector.tensor_copy(
                                psb[bank][:, blk * 512:(blk + 1) * 512],
                                pr[bank][:])
                for bank in range(2):
                    nc.gpsimd.dma_start(d_out[win, bank], psb[bank][:])

    nc.compile()
    _BUILT[key] = nc
    return nc


def _prep_weights(W_in, b_in, W_ih, W_hh, b_ih, b_hh, W_out, b_out):
    f8 = np.float64
    G = W_ih.astype(f8) @ W_in.astype(f8)              # [192, 4]
    c = W_ih.astype(f8) @ b_in.astype(f8) + b_ih       # [192]
    wo = W_out.astype(f8)[0]                           # [64]
    bo = float(b_out[0])
    Wr, Wz, Wn = W_hh[0:64].astype(f8), W_hh[64:128].astype(f8), \
        W_hh[128:192].astype(f8)
    Gr, Gz, Gn = G[0:64], G[64:128], G[128:192]
    cr, cz, cn = c[0:64], c[64:128], c[128:192]
    bhr, bhz, bhn = b_hh[0:64].astype(f8), b_hh[64:128].astype(f8), \
        b_hh[128:192].astype(f8)

    def blockdiag(m):  # [64,64] effective weight -> [128,128] lhsT
        out = np.zeros((128, 128), f8)
        out[0:64, 0:64] = m.T
        out[64:128, 64:128] = m.T
        return out

    def xlhs(Gg, bias):  # [9, 128] x-side lhsT
        out = np.zeros((9, 128), f8)
        out[0:3, 0:64] = Gg[:, 1:4].T
        out[3:6, 64:128] = Gg[:, 1:4].T
        out[6, 0:64] = bias
        out[6, 64:128] = bias
        out[7, 0:64] = Gg[:, 0]
        out[8, 64:128] = Gg[:, 0]
        return out

    def padq(x9, q):  # embed [9,128] lhsT at partition rows 9q:9q+9
        out = np.zeros((128, 128), np.float64)
        out[9 * q:9 * q + 9, :] = x9
        return out

    w = {}
    w["TRH0"] = blockdiag(Wr)
    w["TRH1"] = blockdiag(Wr + np.outer(Gr[:, 0], wo))
    w["TZH0"] = blockdiag(Wz)
    w["TZH1"] = blockdiag(Wz + np.outer(Gz[:, 0], wo))
    w["TIH1"] = blockdiag(np.outer(Gn[:, 0], wo))
    w["THH"] = blockdiag(Wn)
    w["ID128"] = np.eye(128, dtype=f8)
    w["XR0P"] = padq(xlhs(Gr, cr + bhr), 0)
    w["XZ0P"] = padq(xlhs(Gz, cz + bhz), 0)
    w["XI0P"] = padq(xlhs(Gn, cn), 0)
    xr1 = xlhs(Gr, cr + bhr + Gr[:, 0] * bo)
    xz1 = xlhs(Gz, cz + bhz + Gz[:, 0] * bo)
    xi1 = xlhs(Gn, cn + Gn[:, 0] * bo)
    w["XRP"] = np.concatenate([padq(xr1, q) for q in range(12)], axis=1)
    w["XZP"] = np.concatenate([padq(xz1, q) for q in range(12)], axis=1)
    w["XIP"] = np.concatenate([padq(xi1, q) for q in range(12)], axis=1)
    pw = np.zeros((128, 32 * 128), f8)
    for a in range(4):
        for j in range(8):
            k = a * 8 + j
            col = 32 * a + 2 * j
            pw[0:64, k * 128 + col] = wo
            pw[64:128, k * 128 + col + 1] = wo
    w["PW"] = pw
    w["BHHN"] = np.concatenate([bhn, bhn])[:, None]

    import ml_dtypes
    return {k: np.ascontiguousarray(
        v.astype(np.float32 if k == "BHHN" else ml_dtypes.bfloat16))
        for k, v in w.items()}


def kernel(X, H, xn, W_in, b_in, W_ih, W_hh, b_ih, b_hh, W_out, b_out):
    global LAST_RESULTS
    import ml_dtypes
    X = np.asarray(X, np.float32)
    H = np.asarray(H, np.float32)
    xn = np.asarray(xn, np.float32)
    bo = float(np.asarray(b_out)[0])
    wmap = _prep_weights(np.asarray(W_in), np.asarray(b_in), np.asarray(W_ih),
                         np.asarray(W_hh), np.asarray(b_ih), np.asarray(b_hh),
                         np.asarray(W_out), np.asarray(b_out))

    Xs = X[:, T_HIST:T_HIST + T_FC, :, F_IN - 3:F_IN]  # [B, 48, C, 3]

    in_maps = []
    for ci in range(N_CORES):
        bs = slice(ci * B_LOC, (ci + 1) * B_LOC)
        # [t, f, col], col = b*C + c
        r3 = np.transpose(Xs[bs], (1, 3, 0, 2)).reshape(T_FC, 3, NCOLS)
        r6 = r3.reshape(T_FC, 3, ND, 2, 2, 512)  # [t,f,dbl,odd,ab,col]
        xtA = r6[:, :, :, :, 0, :].reshape(T_FC, 3, ND, 1024)
        xtB = r6[:, :, :, :, 1, :].reshape(T_FC, 3, ND, 1024)
        # rows 9q:9q+9 of block t//12 = [xtA(3); xtB(3); ones; xnA; xnB]
        XTD = np.zeros((ND, 4, 128, 1024), ml_dtypes.bfloat16)
        for t in range(T_FC):
            blk, qq = t // 12, t % 12
            XTD[:, blk, 9 * qq:9 * qq + 3] = xtA[t].transpose(1, 0, 2)
            XTD[:, blk, 9 * qq + 3:9 * qq + 6] = xtB[t].transpose(1, 0, 2)
            XTD[:, blk, 9 * qq + 6] = 1.0
        xn6 = xn[bs, :, 0].reshape(ND, 2, 2, 512)
        XTD[:, 0, 7, :] = xn6[:, :, 0, :].reshape(ND, 1024)
        XTD[:, 0, 8, :] = xn6[:, :, 1, :].reshape(ND, 1024)

        h6 = H[bs].reshape(NCOLS, HID).reshape(ND, 2, 2, 512, HID)
        HT = np.empty((ND, 128, 1024), ml_dtypes.bfloat16)
        HT[:, 0:64] = h6[:, :, 0].transpose(0, 3, 1, 2).reshape(ND, 64, 1024)
        HT[:, 64:128] = h6[:, :, 1].transpose(0, 3, 1, 2) \
            .reshape(ND, 64, 1024)

        m = {"XTD": XTD, "HT": np.ascontiguousarray(HT)}
        m.update(wmap)
        in_maps.append(m)

    nc = _build()

    trace = os.environ.get("BASS_KERNEL_TRACE") == "1"
    if trace:
        _register_ntff_hook()
    res = run_bass_kernel_spmd(nc, in_maps, list(range(N_CORES)), trace=trace)
    LAST_RESULTS = res

    out = np.empty((B, T_FC, C, 1), np.float32)
    for ci in range(N_CORES):
        O = np.asarray(res.results[ci]["OUT"], np.float32)
        # [w, bank, a, q(32), blk, col] -> keep q<16 -> [w,bank,a,j,ch,blk,col]
        O7 = O.reshape(4, 2, 4, 32, 6, 512)[:, :, :, 0:16] \
            .reshape(4, 2, 4, 8, 2, 6, 512)
        # -> [blk, j, w, bank, a, ch, col] = [t, row]
        P = O7.transpose(5, 3, 0, 1, 2, 4, 6).reshape(T_FC, NCOLS) + bo
        out[ci * B_LOC:(ci + 1) * B_LOC] = \
            P.reshape(T_FC, B_LOC, C).transpose(1, 0, 2)[..., None]
    return out


def _register_ntff_hook():
    """The agent image's antenv lacks axon_hooks; provide it so trace=True
    can capture NTFF profiles through libaxon_pjrt."""
    import sys
    import types
    if "antenv.axon_hooks" in sys.modules:
        return
    mod = types.ModuleType("antenv.axon_hooks")
    state = {"hook": None}
    mod.set_axon_ntff_profile_hook = lambda h: state.update(hook=h)
    mod.get_axon_ntff_profile_hook = lambda: state["hook"]
    sys.modules["antenv.axon_hooks"] = mod
    try:
        import antenv
        antenv.axon_hooks = mod
    except ImportError:
        pass
    try:
        from trn_agent_boot.trn_boot import _ntff_profile_via_ctypes
        hook = _ntff_profile_via_ctypes("/opt/axon/libaxon_pjrt.so")
        if hook is not None:
            mod.set_axon_ntff_profile_hook(hook)
    except Exception as e:  # pragma: no cover
        print(f"NTFF hook registration failed: {e}")
    # No artifact bucket in this sandbox; keep profiles local.
    import concourse.bass_utils as bu
    bu.upload_artifacts = lambda tmpdir: f"file://{tmpdir}"


# revision 7
# speedup vs baseline: 2.4120x; 1.0001x over previous
"""Trainium2 Bass kernel for nn_Decoder (GRU decoder, B=64, T_FC=48, C=4096, HID=64).

Strategy (v2)
-------------
Data-parallel over batch: 8 cores x 8 batch rows -> 32768 independent GRU
"columns" per core.  Columns are processed in GROUPS of 1024: chunk A
(cols 0:512) occupies partitions 0:64, chunk B (cols 512:1024) partitions
64:128, so every DVE/ACT op runs with all 128 partitions busy.

Host algebra folds fc_in and the autoregressive x_prev feedback into the
gate weights (t>=1):
    G = W_ih @ W_in                       [192, 4]
    pre_g   = (W_hh_g + G_g0 wo^T) h + G_g,1:4 xt + bias_g     (g in r,z)
    i_n     = (G_n0 wo^T) h + G_n,1:4 xt + bias_n
    h_n     = W_hh_n h            (+ b_hh_n via scalar_tensor_tensor)
    n = tanh(i_n + r*h_n);  h' = n + z*(h - n);  pred = wo @ h' (+ b_out host)

Per group-step: 9 back-to-back N=512 bf16 matmuls (block-diagonal K=128
h-side, K=9 x-side incl. bias/aux rows, identity-matmul accumulating
r*h_n into the i_n psum, sparse pred matmul parking 8 steps of preds in
one psum bank), 1 sigmoid [128,1024], 1 tanh [128,512], and ~4.5 DVE ops
(update ops run per double-group at [128,1024]).  Dense MM stream keeps
the PE at full clock (216ns/MM vs 604ns in the dependency-choked v1).
"""

import os

import numpy as np

import concourse.bass as bass
import concourse.mybir as mybir
import concourse.tile as tile
from concourse import bacc
from concourse.bass_utils import run_bass_kernel_spmd

F32 = mybir.dt.float32
BF16 = mybir.dt.bfloat16
AF = mybir.ActivationFunctionType
ALU = mybir.AluOpType

B, T_HIST, T_FC, C, F_IN, HID = 64, 24, 48, 4096, 8, 64
N_CORES = 8
B_LOC = B // N_CORES
NCOLS = B_LOC * C          # 32768 columns per core
NG = 32                    # groups of 1024 columns
ND = 16                    # double-groups
W_GROUPS = 8               # groups per window (4 double-groups)
XQ = 12                    # xt steps packed per [128,1024] tile

_BUILT = {}
LAST_RESULTS = None  # BassKernelResults of the most recent run (for test.py)

W_SHAPES = {
    "TRH0": [128, 128], "TRH1": [128, 128],
    "TZH0": [128, 128], "TZH1": [128, 128],
    "TIH1": [128, 128], "THH": [128, 128], "ID128": [128, 128],
    "XR0P": [128, 128], "XZ0P": [128, 128], "XI0P": [128, 128],
    "XRP": [128, 12 * 128], "XZP": [128, 12 * 128], "XIP": [128, 12 * 128],
    "PW": [128, 32 * 128],
    "BHHN": [128, 1],
}


def _build():
    key = "v2"
    if key in _BUILT:
        return _BUILT[key]

    nc = bacc.Bacc("TRN2", target_bir_lowering=False, debug=False,
                   num_devices=N_CORES)

    d_xtd = nc.dram_tensor("XTD", [ND, 4, 128, 1024], BF16,
                           kind="ExternalInput").ap()
    d_ht = nc.dram_tensor("HT", [ND, 128, 1024], BF16,
                          kind="ExternalInput").ap()
    d_w = {name: nc.dram_tensor(name, shape,
                                F32 if name == "BHHN" else BF16,
                                kind="ExternalInput").ap()
           for name, shape in W_SHAPES.items()}
    # preds: [window, bank, 128, 6*512]; row = 32*a + 2*j + chunk
    d_out = nc.dram_tensor("OUT", [NG // W_GROUPS, 2, 128, 3072], BF16,
                           kind="ExternalOutput").ap()

    with tile.TileContext(nc) as tc:
        with (
            tc.tile_pool(name="wpool", bufs=1) as wpool,
            tc.tile_pool(name="xpool", bufs=1) as xpool,
            tc.tile_pool(name="hpool", bufs=1) as hpool,
            tc.tile_pool(name="spool", bufs=1) as spool,
            tc.tile_pool(name="pspool", bufs=1, space="PSUM") as pspool,
        ):
            w = {}
            for name, ap in d_w.items():
                wt = wpool.tile(list(ap.shape), ap.dtype, name=f"w_{name}")
                nc.gpsimd.dma_start(wt[:], ap[:])
                w[name] = wt

            def PWk(a, j):
                k = a * 8 + j
                return w["PW"][:, k * 128:(k + 1) * 128]

            def XWq(name, q):
                return w[name][:, q * 128:(q + 1) * 128]

            for win in range(NG // W_GROUPS):
                Hd = {}
                Sd = {}
                NTd = {}
                xtb = {}
                for d in range(4):
                    dbl = win * 4 + d
                    ht = hpool.tile([128, 2, 512], BF16, tag=f"H{d}",
                                    bufs=2, name="ht")
                    nc.gpsimd.dma_start(ht[:], d_ht[dbl])
                    Hd[d] = ht
                psb = {}
                for bank in range(2):
                    psb[bank] = spool.tile([128, 3072], BF16,
                                           tag=f"psb{bank}", bufs=2,
                                           name="psb")
                pr = {}
                for t in range(T_FC):
                    if t % 8 == 0:
                        pr[0] = pspool.tile([128, 512], F32, tag="pra",
                                            bufs=1, name="pra")
                        pr[1] = pspool.tile([128, 512], F32, tag="prb",
                                            bufs=1, name="prb")
                    trh = w["TRH1"] if t else w["TRH0"]
                    tzh = w["TZH1"] if t else w["TZH0"]
                    q = t % XQ
                    xr = w["XR0P"] if t == 0 else XWq("XRP", q)
                    xz = w["XZ0P"] if t == 0 else XWq("XZP", q)
                    xi = w["XI0P"] if t == 0 else XWq("XIP", q)
                    for gi in range(W_GROUPS):
                        d, j2 = gi // 2, gi % 2
                        dbl = win * 4 + d
                        if t % XQ == 0 and j2 == 0:
                            xt_ = xpool.tile([128, 1024], BF16,
                                             tag=f"xt{d}", bufs=2, name="xt_")
                            nc.gpsimd.dma_start(
                                xt_[:], d_xtd[dbl, t // XQ])
                            xtb[d] = xt_
                        xts = xtb[d][:, j2 * 512:(j2 + 1) * 512]
                        hs = Hd[d][:, j2, :]

                        g = pspool.tile([128, 1024], F32, tag="gates",
                                        bufs=3, name="g")
                        nc.tensor.matmul(g[:, 0:512], trh[:], hs,
                                         start=True, stop=False)
                        nc.tensor.matmul(g[:, 0:512], xr[:], xts,
                                         start=False, stop=True)
                        nc.tensor.matmul(g[:, 512:1024], tzh[:], hs,
                                         start=True, stop=False)
                        nc.tensor.matmul(g[:, 512:1024], xz[:], xts,
                                         start=False, stop=True)

                        if j2 == 0:
                            Sd[d] = spool.tile([128, 2, 1024], BF16,
                                               tag=f"S{d}", bufs=3, name="S")
                            NTd[d] = spool.tile([128, 2, 512], BF16,
                                                tag=f"NT{d}", bufs=3,
                                                name="NT")
                        nc.scalar.activation(Sd[d][:, j2, :], g[:],
                                             AF.Sigmoid)

                        # phase 2: reuse gate banks for [i_n | h_n]
                        if t:
                            nc.tensor.matmul(g[:, 0:512], w["TIH1"][:], hs,
                                             start=True, stop=False)
                            nc.tensor.matmul(g[:, 0:512], xi[:], xts,
                                             start=False, stop=False)
                        else:
                            nc.tensor.matmul(g[:, 0:512], xi[:], xts,
                                             start=True, stop=False)
                        nc.tensor.matmul(g[:, 512:1024], w["THH"][:], hs,
                                         start=True, stop=True)

                        rhn = spool.tile([128, 512], BF16, tag="rhn",
                                         bufs=4, name="rhn")
                        nc.vector.scalar_tensor_tensor(
                            rhn[:], g[:, 512:1024], w["BHHN"][:],
                            Sd[d][:, j2, 0:512], op0=ALU.add, op1=ALU.mult)
                        nc.tensor.matmul(g[:, 0:512], w["ID128"][:], rhn[:],
                                         start=False, stop=True)
                        nc.scalar.activation(NTd[d][:, j2, :], g[:, 0:512],
                                             AF.Tanh)

                        if j2 == 1:
                            hm = spool.tile([128, 2, 512], BF16,
                                            tag=f"HM{d}", bufs=3, name="hm")
                            nc.vector.tensor_tensor(hm[:], Hd[d][:],
                                                    NTd[d][:],
                                                    op=ALU.subtract)
                            zt = spool.tile([128, 2, 512], BF16,
                                            tag=f"ZT{d}", bufs=3, name="zt")
                            nc.vector.tensor_tensor(zt[:],
                                                    Sd[d][:, :, 512:1024],
                                                    hm[:], op=ALU.mult)
                            nc.vector.tensor_tensor(Hd[d][:], NTd[d][:],
                                                    zt[:], op=ALU.add)
                            # preds of h^{t+1} for both groups of the pair
                            for jj in (0, 1):
                                gidx = 2 * d + jj
                                a, bank = gidx % 4, gidx // 4
                                nc.tensor.matmul(
                                    pr[bank][:], PWk(a, t % 8),
                                    Hd[d][:, jj, :],
                                    start=(t % 8 == 0 and a == 0),
                                    stop=(t % 8 == 7 and a == 3),
                                    skip_group_check=True)
                    if t % 8 == 7:
                        blk = t // 8
                        for bank in range(2):
                            nc.vector.tensor_copy(
                                psb[bank][:, blk * 512:(blk + 1) * 512],
                                pr[bank][:])
                for bank in range(2):
                    nc.gpsimd.dma_start(d_out[win, bank], psb[bank][:])

    nc.compile()
    _BUILT[key] = nc
    return nc


def _prep_weights(W_in, b_in, W_ih, W_hh, b_ih, b_hh, W_out, b_out):
    f8 = np.float64
    G = W_ih.astype(f8) @ W_in.astype(f8)              # [192, 4]
    c = W_ih.astype(f8) @ b_in.astype(f8) + b_ih       # [192]
    wo = W_out.astype(f8)[0]                           # [64]
    bo = float(b_out[0])
    Wr, Wz, Wn = W_hh[0:64].astype(f8), W_hh[64:128].astype(f8), \
        W_hh[128:192].astype(f8)
    Gr, Gz, Gn = G[0:64], G[64:128], G[128:192]
    cr, cz, cn = c[0:64], c[64:128], c[128:192]
    bhr, bhz, bhn = b_hh[0:64].astype(f8), b_hh[64:128].astype(f8), \
        b_hh[128:192].astype(f8)

    def blockdiag(m):  # [64,64] effective weight -> [128,128] lhsT
        out = np.zeros((128, 128), f8)
        out[0:64, 0:64] = m.T
        out[64:128, 64:128] = m.T
        return out

    def xlhs(Gg, bias):  # [9, 128] x-side lhsT
        out = np.zeros((9, 128), f8)
        out[0:3, 0:64] = Gg[:, 1:4].T
        out[3:6, 64:128] = Gg[:, 1:4].T
        out[6, 0:64] = bias
        out[6, 64:128] = bias
        out[7, 0:64] = Gg[:, 0]
        out[8, 64:128] = Gg[:, 0]
        return out

    def padq(x9, q):  # embed [9,128] lhsT at partition rows 9q:9q+9
        out = np.zeros((128, 128), np.float64)
        out[9 * q:9 * q + 9, :] = x9
        return out

    w = {}
    w["TRH0"] = blockdiag(Wr)
    w["TRH1"] = blockdiag(Wr + np.outer(Gr[:, 0], wo))
    w["TZH0"] = blockdiag(Wz)
    w["TZH1"] = blockdiag(Wz + np.outer(Gz[:, 0], wo))
    w["TIH1"] = blockdiag(np.outer(Gn[:, 0], wo))
    w["THH"] = blockdiag(Wn)
    w["ID128"] = np.eye(128, dtype=f8)
    w["XR0P"] = padq(xlhs(Gr, cr + bhr), 0)
    w["XZ0P"] = padq(xlhs(Gz, cz + bhz), 0)
    w["XI0P"] = padq(xlhs(Gn, cn), 0)
    xr1 = xlhs(Gr, cr + bhr + Gr[:, 0] * bo)
    xz1 = xlhs(Gz, cz + bhz + Gz[:, 0] * bo)
    xi1 = xlhs(Gn, cn + Gn[:, 0] * bo)
    w["XRP"] = np.concatenate([padq(xr1, q) for q in range(12)], axis=1)
    w["XZP"] = np.concatenate([padq(xz1, q) for q in range(12)], axis=1)
    w["XIP"] = np.concatenate([padq(xi1, q) for q in range(12)], axis=1)
    pw = np.zeros((128, 32 * 128), f8)
    for a in range(4):
        for j in range(8):
            k = a * 8 + j
            col = 32 * a + 2 * j
            pw[0:64, k * 128 + col] = wo
            pw[64:128, k * 128 + col + 1] = wo
    w["PW"] = pw
    w["BHHN"] = np.concatenate([bhn, bhn])[:, None]

    import ml_dtypes
    return {k: np.ascontiguousarray(
        v.astype(np.float32 if k == "BHHN" else ml_dtypes.bfloat16))
        for k, v in w.items()}


def kernel(X, H, xn, W_in, b_in, W_ih, W_hh, b_ih, b_hh, W_out, b_out):
    global LAST_RESULTS
    import ml_dtypes
    X = np.asarray(X, np.float32)
    H = np.asarray(H, np.float32)
    xn = np.asarray(xn, np.float32)
    bo = float(np.asarray(b_out)[0])
    wmap = _prep_weights(np.asarray(W_in), np.asarray(b_in), np.asarray(W_ih),
                         np.asarray(W_hh), np.asarray(b_ih), np.asarray(b_hh),
                         np.asarray(W_out), np.asarray(b_out))

    Xs = X[:, T_HIST:T_HIST + T_FC, :, F_IN - 3:F_IN]  # [B, 48, C, 3]

    in_maps = []
    for ci in range(N_CORES):
        bs = slice(ci * B_LOC, (ci + 1) * B_LOC)
        # [t, f, col], col = b*C + c
        r3 = np.transpose(Xs[bs], (1, 3, 0, 2)).reshape(T_FC, 3, NCOLS)
        r6 = r3.reshape(T_FC, 3, ND, 2, 2, 512)  # [t,f,dbl,odd,ab,col]
        xtA = r6[:, :, :, :, 0, :].reshape(T_FC, 3, ND, 1024)
        xtB = r6[:, :, :, :, 1, :].reshape(T_FC, 3, ND, 1024)
        # rows 9q:9q+9 of block t//12 = [xtA(3); xtB(3); ones; xnA; xnB]
        XTD = np.zeros((ND, 4, 128, 1024), ml_dtypes.bfloat16)
        for t in range(T_FC):
            blk, qq = t // 12, t % 12
            XTD[:, blk, 9 * qq:9 * qq + 3] = xtA[t].transpose(1, 0, 2)
            XTD[:, blk, 9 * qq + 3:9 * qq + 6] = xtB[t].transpose(1, 0, 2)
            XTD[:, blk, 9 * qq + 6] = 1.0
        xn6 = xn[bs, :, 0].reshape(ND, 2, 2, 512)
        XTD[:, 0, 7, :] = xn6[:, :, 0, :].reshape(ND, 1024)
        XTD[:, 0, 8, :] = xn6[:, :, 1, :].reshape(ND, 1024)

        h6 = H[bs].reshape(NCOLS, HID).reshape(ND, 2, 2, 512, HID)
        HT = np.empty((ND, 128, 1024), ml_dtypes.bfloat16)
        HT[:, 0:64] = h6[:, :, 0].transpose(0, 3, 1, 2).reshape(ND, 64, 1024)
        HT[:, 64:128] = h6[:, :, 1].transpose(0, 3, 1, 2) \
            .reshape(ND, 64, 1024)

        m = {"XTD": XTD, "HT": np.ascontiguousarray(HT)}
        m.update(wmap)
        in_maps.append(m)

    nc = _build()

    trace = os.environ.get("BASS_KERNEL_TRACE") == "1"
    if trace:
        _register_ntff_hook()
    res = run_bass_kernel_spmd(nc, in_maps, list(range(N_CORES)), trace=trace)
    LAST_RESULTS = res

    out = np.empty((B, T_FC, C, 1), np.float32)
    for ci in range(N_CORES):
        O = np.asarray(res.results[ci]["OUT"], np.float32)
        # [w, bank, a, q(32), blk, col] -> keep q<16 -> [w,bank,a,j,ch,blk,col]
        O7 = O.reshape(4, 2, 4, 32, 6, 512)[:, :, :, 0:16] \
            .reshape(4, 2, 4, 8, 2, 6, 512)
        # -> [blk, j, w, bank, a, ch, col] = [t, row]
        P = O7.transpose(5, 3, 0, 1, 2, 4, 6).reshape(T_FC, NCOLS) + bo
        out[ci * B_LOC:(ci + 1) * B_LOC] = \
            P.reshape(T_FC, B_LOC, C).transpose(1, 0, 2)[..., None]
    return out


def _register_ntff_hook():
    """The agent image's antenv lacks axon_hooks; provide it so trace=True
    can capture NTFF profiles through libaxon_pjrt."""
    import sys
    import types
    if "antenv.axon_hooks" in sys.modules:
        return
    mod = types.ModuleType("antenv.axon_hooks")
    state = {"hook": None}
    mod.set_axon_ntff_profile_hook = lambda h: state.update(hook=h)
    mod.get_axon_ntff_profile_hook = lambda: state["hook"]
    sys.modules["antenv.axon_hooks"] = mod
    try:
        import antenv
        antenv.axon_hooks = mod
    except ImportError:
        pass
    try:
        from trn_agent_boot.trn_boot import _ntff_profile_via_ctypes
        hook = _ntff_profile_via_ctypes("/opt/axon/libaxon_pjrt.so")
        if hook is not None:
            mod.set_axon_ntff_profile_hook(hook)
    except Exception as e:  # pragma: no cover
        print(f"NTFF hook registration failed: {e}")
    # No artifact bucket in this sandbox; keep profiles local.
    import concourse.bass_utils as bu
    bu.upload_artifacts = lambda tmpdir: f"file://{tmpdir}"
